# revision 3
# baseline (speedup 1.0000x reference)
"""Fused DDPM dynamic-conv kernel for TRN2 (8 NeuronCores).

Math (reference):
  kernels = einsum('nchw,oc->nohw', y, gen_w) + gen_b        # o = d*576 + c*9 + t
  r_d     = sum_t kernels[d,c,t] * shift(x, tap t, dil d)    # d in {1,3,5}
  out     = conv3x3([x, r1, r3, r5], fuse_w) + fuse_b

Sharding: 8 cores = 4 batches x 2 H-halves (48 output rows each).
Per core, the 50 kern rows (48 + 1 halo each side) are split into two
26-row blocks (2-row overlap) packed on SBUF partitions: p = 64*blk + c.

Engine split (vs. the all-PE/DVE baseline):
  PE  : gen matmuls (f32r) + fuse conv matmuls (bf16).  No identity
        tap-accumulation matmuls.
  ACT : evicts kern PSUM->SBUF bf16 with the gen bias fused
        (activation Identity + per-partition bias), and the fuse
        PSUM->SBUF eviction with fuse_b.
  DVE : tap products in bf16 (tensor_tensor mult runs in 2x mode for
        2-byte packed operands) + part of the add-tree.
  Pool: some evicts (taps 7,8) + part of the add-tree.
Tap accumulation is a 4-level pairwise tree in bf16; racc is bf16 so
the fuse matmuls stream it at 1 cycle/row like f32r.
"""

import numpy as np

K = 3
NB, C, H, W = 4, 64, 96, 96
NCORES = 8
HH = 48            # output rows per core
BLK = 26           # kern rows per block (24 out + 1 halo each side)
XR = BLK + 10      # x rows per block (halo 5 each side for dil 5)
WP = W + 10        # padded width for x
RW = W + 2         # padded width for racc
DILS = (1, 3, 5)
KCHUNKS = ((0, 6), (6, 10), (16, 10))   # kern-row chunks (start, nrows)
FCH = tuple((1 + 3 * i, 3) for i in range(8))  # fuse out-row chunks
EV_POOL_TAPS = (7, 8)  # evict units on Pool; rest on ACT

_built = None


def _build():
    import concourse.mybir as mybir
    from concourse import bacc
    from concourse.tile import TileContext

    f32 = mybir.dt.float32
    f32r = mybir.dt.float32r
    bf16 = mybir.dt.bfloat16
    add = mybir.AluOpType.add
    mult = mybir.AluOpType.mult
    ident = mybir.ActivationFunctionType.Identity

    nc = bacc.Bacc()
    xh = nc.dram_tensor("xh", [C, 60, WP], bf16, kind="ExternalInput")
    yh = nc.dram_tensor("yh", [C, 50, W], f32r, kind="ExternalInput")
    wg = nc.dram_tensor("wg", [128, 27 * 128], f32r, kind="ExternalInput")
    gb = nc.dram_tensor("gb", [128, 27], f32, kind="ExternalInput")
    fw = nc.dram_tensor("fw", [128, 9 * 4 * 128], bf16, kind="ExternalInput")
    fb = nc.dram_tensor("fb", [128, 1], f32, kind="ExternalInput")
    rm = nc.dram_tensor("rm", [128, 2], f32, kind="ExternalInput")
    out = nc.dram_tensor("out", [C, HH, W], f32, kind="ExternalOutput")

    with TileContext(nc) as tc:
        with (
            tc.tile_pool(name="const", bufs=1) as cpool,
            tc.tile_pool(name="ks", bufs=4) as kspool,
            tc.tile_pool(name="pa", bufs=2) as papool,
            tc.tile_pool(name="sa", bufs=2) as sapool,
            tc.tile_pool(name="kpsum", bufs=2, space="PSUM") as kpool,
            tc.tile_pool(name="fpsum", bufs=4, space="PSUM") as fpool,
        ):
            xpad = cpool.tile([128, XR, WP], bf16)
            ysb = cpool.tile([128, BLK * W], f32r)
            wgsb = cpool.tile([128, 27 * 128], f32r)
            gbsb = cpool.tile([128, 27], f32)
            fwsb = cpool.tile([128, 9 * 4 * 128], bf16)
            fbsb = cpool.tile([128, 1], f32)
            rmsb = cpool.tile([128, 2], f32)
            racc = cpool.tile([128, 3, BLK, RW], bf16)
            osb = cpool.tile([128, 24, W], f32)

            # zero the 1-col borders of racc (cols 0 and 97)
            nc.gpsimd.memset(racc[:, :, :, 0:RW:RW - 1], 0.0)
            # loads in first-use order
            ys3 = ysb[:].rearrange("p (r w) -> p r w", r=BLK)
            nc.sync.dma_start(out=ys3[0:64, 0:6, :], in_=yh[:, 0:6, :])
            nc.sync.dma_start(out=ys3[64:128, 0:6, :], in_=yh[:, 24:30, :])
            nc.sync.dma_start(out=wgsb[:, 0:1152], in_=wg[:, 0:1152])
            nc.sync.dma_start(out=gbsb[:, :], in_=gb[:, :])
            nc.sync.dma_start(out=xpad[0:64, 0:16, :], in_=xh[:, 0:16, :])
            nc.sync.dma_start(out=xpad[64:128, 0:16, :], in_=xh[:, 24:40, :])
            nc.sync.dma_start(out=wgsb[:, 1152:], in_=wg[:, 1152:])
            nc.sync.dma_start(out=ys3[0:64, 6:BLK, :], in_=yh[:, 6:BLK, :])
            nc.sync.dma_start(out=ys3[64:128, 6:BLK, :], in_=yh[:, 30:50, :])
            nc.sync.dma_start(out=xpad[0:64, 16:XR, :], in_=xh[:, 16:XR, :])
            nc.sync.dma_start(out=xpad[64:128, 16:XR, :], in_=xh[:, 40:24 + XR, :])
            nc.sync.dma_start(out=rmsb[:, :], in_=rm[:, :])
            nc.sync.dma_start(out=fwsb[:, 0:2304], in_=fw[:, 0:2304])
            nc.sync.dma_start(out=fwsb[:, 2304:], in_=fw[:, 2304:])
            nc.sync.dma_start(out=fbsb[:, :], in_=fb[:, :])

            # ---- fuse conv: per out-chunk, 4 parts of 9 matmuls (one per
            # input group), gated on racc[g-1]; part 3 evicts + stores.
            fuse_state = {}
            parts_done = {o0: 0 for (o0, _) in FCH}

            def fuse_part(o0, g):
                if g == 0:
                    fp = fpool.tile([128, 3 * W], f32, tag="fp")
                    fuse_state[o0] = fp
                fp = fuse_state[o0]
                fpv = fp[:].rearrange("p (r w) -> p r w", r=3)
                for ij in range(9):
                    di, dj = ij // 3 - 1, ij % 3 - 1
                    if g == 0:
                        rhs = xpad[:, o0 + di + 5:o0 + di + 8, 5 + dj:5 + dj + W]
                    else:
                        rhs = racc[:, g - 1, o0 + di:o0 + di + 3, 1 + dj:1 + dj + W]
                    nc.tensor.matmul(
                        fpv, fwsb[:, (ij * 4 + g) * 128:(ij * 4 + g + 1) * 128],
                        rhs, start=(g == 0 and ij == 0), stop=(g == 3 and ij == 8),
                    )
                if g == 3:
                    nc.scalar.activation(
                        osb[:, o0 - 1:o0 + 2, :], fpv, ident, bias=fbsb[:, 0:1])
                    del fuse_state[o0]
                    nc.sync.dma_start(out=out[:, o0 - 1:o0 + 2, :],
                                      in_=osb[0:64, o0 - 1:o0 + 2, :])
                    nc.sync.dma_start(out=out[:, 23 + o0:26 + o0, :],
                                      in_=osb[64:128, o0 - 1:o0 + 2, :])
                parts_done[o0] = g + 1

            def pump_fuse(ci, dd):
                r_end = KCHUNKS[ci][0] + KCHUNKS[ci][1]
                prev_end = KCHUNKS[ci - 1][0] + KCHUNKS[ci - 1][1] if ci else 0
                for o0, nr in FCH:
                    if o0 + nr + 1 > r_end:
                        break
                    if o0 + nr + 1 <= prev_end:
                        # covered by an earlier chunk: all deps ready
                        while parts_done[o0] < 4:
                            fuse_part(o0, parts_done[o0])
                    else:
                        # newly covered: part g needs racc[g-1] (done at dil g-1)
                        while parts_done[o0] <= dd:
                            fuse_part(o0, parts_done[o0])

            for ci, (r0, nrc) in enumerate(KCHUNKS):
                nh = nrc // 2  # rows per gen matmul (one PSUM bank each)
                for dd, d in enumerate(DILS):
                    pa = papool.tile([128, 9, 10, W], bf16, tag="pa")
                    sa = sapool.tile([128, 7, 10, W], bf16, tag="sa")
                    for t in range(9):
                        di, dj = t // 3 - 1, t % 3 - 1
                        dt = dd * 9 + t
                        kp = kpool.tile([128, 2, 512], f32, tag="kp")
                        for k in (0, 1):
                            nc.tensor.matmul(
                                kp[:, k, 0:nh * W],
                                wgsb[:, dt * 128:(dt + 1) * 128],
                                ysb[:, (r0 + k * nh) * W:(r0 + (k + 1) * nh) * W],
                                start=True, stop=True,
                            )
                        ks = kspool.tile([128, 10, W], bf16, tag="ks")
                        kpv = kp[:, :, 0:nh * W]
                        ksv = ks[:, 0:nrc, :].rearrange("p (b r) w -> p b (r w)", b=2)
                        if t in EV_POOL_TAPS:
                            nc.gpsimd.tensor_scalar(
                                ksv, kpv, gbsb[:, dt:dt + 1], None, add)
                        else:
                            nc.scalar.activation(
                                ksv, kpv, ident, bias=gbsb[:, dt:dt + 1])
                        x0 = r0 + di * d + 5
                        nc.vector.tensor_tensor(
                            pa[:, t, 0:nrc, :], ks[:, 0:nrc, :],
                            xpad[:, x0:x0 + nrc, 5 + dj * d:5 + dj * d + W], mult)
                    # 4-level pairwise tap-accumulation tree -> racc (bf16)
                    nc.gpsimd.tensor_tensor(
                        sa[:, 0:2, 0:nrc, :], pa[:, 0:4:2, 0:nrc, :],
                        pa[:, 1:4:2, 0:nrc, :], add)
                    nc.vector.tensor_tensor(
                        sa[:, 2:4, 0:nrc, :], pa[:, 4:8:2, 0:nrc, :],
                        pa[:, 5:8:2, 0:nrc, :], add)
                    nc.gpsimd.tensor_tensor(
                        sa[:, 4:6, 0:nrc, :], sa[:, 0:4:2, 0:nrc, :],
                        sa[:, 1:4:2, 0:nrc, :], add)
                    nc.vector.tensor_tensor(
                        sa[:, 6, 0:nrc, :], sa[:, 4, 0:nrc, :],
                        sa[:, 5, 0:nrc, :], add)
                    nc.vector.tensor_tensor(
                        racc[:, dd, r0:r0 + nrc, 1:1 + W], sa[:, 6, 0:nrc, :],
                        pa[:, 8, 0:nrc, :], add)
                    # zero out-of-image halo rows (reference zero-pads cat)
                    if r0 == 0:
                        nc.vector.tensor_scalar_mul(
                            racc[:, dd, 0, 1:1 + W], racc[:, dd, 0, 1:1 + W],
                            rmsb[:, 0:1])
                    elif r0 + nrc == BLK:
                        nc.vector.tensor_scalar_mul(
                            racc[:, dd, BLK - 1, 1:1 + W],
                            racc[:, dd, BLK - 1, 1:1 + W], rmsb[:, 1:2])
                    pump_fuse(ci, dd)
            for o0, _ in FCH:
                while parts_done[o0] < 4:
                    fuse_part(o0, parts_done[o0])
    nc.finalize()
    return nc


def _prep_inputs(x, y, gen_w, gen_b, fuse_w, fuse_b):
    import ml_dtypes
    bf16 = ml_dtypes.bfloat16
    # generator weights: W_dt[c', c] = gen_w[d*576 + c*9 + t, c'],
    # block-diagonal over the two H-blocks.
    w3 = gen_w.reshape(3, 64, 9, 64).transpose(3, 0, 2, 1).reshape(64, 27, 64)
    wgh = np.zeros((128, 27, 128), np.float32)
    wgh[0:64, :, 0:64] = w3
    wgh[64:128, :, 64:128] = w3
    wgh = np.ascontiguousarray(wgh.reshape(128, 27 * 128))
    gbh = gen_b.reshape(3, 64, 9).transpose(1, 0, 2).reshape(64, 27)
    gbh = np.ascontiguousarray(np.concatenate([gbh, gbh], 0))
    # fuse weights: [k, ij, g, o] block-diagonal
    f3 = fuse_w.transpose(1, 2, 3, 0).reshape(4, 64, 9, 64).transpose(1, 2, 0, 3)
    fwh2 = np.zeros((128, 9, 4, 128), np.float32)
    fwh2[0:64, :, :, 0:64] = f3
    fwh2[64:128, :, :, 64:128] = f3
    fwh = np.ascontiguousarray(fwh2.reshape(128, 9 * 4 * 128).astype(bf16))
    fbh = np.ascontiguousarray(np.concatenate([fuse_b, fuse_b])[:, None])
    xp = np.pad(x, ((0, 0), (0, 0), (6, 6), (5, 5))).astype(bf16)
    yp = np.pad(y, ((0, 0), (0, 0), (1, 1), (0, 0)))
    maps = []
    for c in range(NCORES):
        n, half = c // 2, c % 2
        h0 = HH * half
        rmh = np.ones((128, 2), np.float32)
        if half == 0:
            rmh[0:64, 0] = 0.0     # block A row 0 = global row -1
        else:
            rmh[64:128, 1] = 0.0   # block B row 25 = global row 96
        maps.append({
            "xh": np.ascontiguousarray(xp[n, :, h0:h0 + 60, :]),
            "yh": np.ascontiguousarray(yp[n, :, h0:h0 + 50, :]),
            "wg": wgh, "gb": gbh, "fw": fwh, "fb": fbh, "rm": rmh,
        })
    return maps


def kernel(x, y, gen_w, gen_b, fuse_w, fuse_b):
    global _built
    from concourse.bass_utils import run_bass_kernel_spmd

    x = np.asarray(x, np.float32)
    y = np.asarray(y, np.float32)
    gen_w = np.asarray(gen_w, np.float32)
    gen_b = np.asarray(gen_b, np.float32)
    fuse_w = np.asarray(fuse_w, np.float32)
    fuse_b = np.asarray(fuse_b, np.float32)

    if _built is None:
        _built = _build()
    maps = _prep_inputs(x, y, gen_w, gen_b, fuse_w, fuse_b)
    res = run_bass_kernel_spmd(_built, maps, list(range(NCORES)))
    outf = np.empty((NB, C, H, W), np.float32)
    for c in range(NCORES):
        n, half = c // 2, c % 2
        outf[n, :, HH * half:HH * half + HH, :] = res.results[c]["out"]
    return outf


# revision 5
# speedup vs baseline: 1.0724x; 1.0724x over previous
"""Fused DDPM dynamic-conv kernel for TRN2 (8 NeuronCores).

Math (reference):
  kernels = einsum('nchw,oc->nohw', y, gen_w) + gen_b        # o = d*576 + c*9 + t
  r_d     = sum_t kernels[d,c,t] * shift(x, tap t, dil d)    # d in {1,3,5}
  out     = conv3x3([x, r1, r3, r5], fuse_w) + fuse_b

Sharding: 8 cores = 4 batches x 2 H-halves (48 output rows each).
Per core, the 50 kern rows (48 + 1 halo each side) are split into two
26-row blocks (2-row overlap) packed on SBUF partitions: p = 64*blk + c.

Engine split (vs. the all-PE/DVE baseline):
  PE  : gen matmuls (f32r) + fuse conv matmuls (bf16).  No identity
        tap-accumulation matmuls.
  ACT : evicts kern PSUM->SBUF bf16 with the gen bias fused
        (activation Identity + per-partition bias), and the fuse
        PSUM->SBUF eviction with fuse_b.
  DVE : tap products in bf16 (tensor_tensor mult runs in 2x mode for
        2-byte packed operands) + part of the add-tree.
  Pool: some evicts (taps 7,8) + part of the add-tree.
Tap accumulation is a 4-level pairwise tree in bf16; racc is bf16 so
the fuse matmuls stream it at 1 cycle/row like f32r.
"""

import numpy as np

K = 3
NB, C, H, W = 4, 64, 96, 96
NCORES = 8
HH = 48            # output rows per core
BLK = 26           # kern rows per block (24 out + 1 halo each side)
XR = BLK + 10      # x rows per block (halo 5 each side for dil 5)
WP = W + 10        # padded width for x
RW = W + 2         # padded width for racc
DILS = (1, 3, 5)
KCHUNKS = ((0, 6), (6, 10), (16, 10))   # kern-row chunks (start, nrows)
FCH = tuple((1 + 3 * i, 3) for i in range(8))  # fuse out-row chunks
EV_POOL_TAPS = (7, 8)  # evict units on Pool; rest on ACT

_built = None


def _build():
    import concourse.mybir as mybir
    from concourse import bacc
    from concourse.tile import TileContext

    f32 = mybir.dt.float32
    f32r = mybir.dt.float32r
    bf16 = mybir.dt.bfloat16
    add = mybir.AluOpType.add
    mult = mybir.AluOpType.mult
    ident = mybir.ActivationFunctionType.Identity

    nc = bacc.Bacc()
    xh = nc.dram_tensor("xh", [C, 60, WP], bf16, kind="ExternalInput")
    yh = nc.dram_tensor("yh", [C, 50, W], f32r, kind="ExternalInput")
    wg = nc.dram_tensor("wg", [128, 27 * 128], f32r, kind="ExternalInput")
    gb = nc.dram_tensor("gb", [128, 27], f32, kind="ExternalInput")
    fw = nc.dram_tensor("fw", [128, 9 * 4 * 128], bf16, kind="ExternalInput")
    fb = nc.dram_tensor("fb", [128, 1], f32, kind="ExternalInput")
    rm = nc.dram_tensor("rm", [128, 2], f32, kind="ExternalInput")
    out = nc.dram_tensor("out", [C, HH, W], f32, kind="ExternalOutput")

    with TileContext(nc) as tc:
        with (
            tc.tile_pool(name="const", bufs=1) as cpool,
            tc.tile_pool(name="ks", bufs=4) as kspool,
            tc.tile_pool(name="pa", bufs=2) as papool,
            tc.tile_pool(name="sa", bufs=2) as sapool,
            tc.tile_pool(name="kpsum", bufs=2, space="PSUM") as kpool,
            tc.tile_pool(name="fpsum", bufs=4, space="PSUM") as fpool,
        ):
            xpad = cpool.tile([128, XR, WP], bf16)
            ysb = cpool.tile([128, BLK * W], f32r)
            wgsb = cpool.tile([128, 27 * 128], f32r)
            gbsb = cpool.tile([128, 27], f32)
            fwsb = cpool.tile([128, 9 * 4 * 128], bf16)
            fbsb = cpool.tile([128, 1], f32)
            rmsb = cpool.tile([128, 2], f32)
            racc = cpool.tile([128, 3, BLK, RW], bf16)
            osb = cpool.tile([128, 24, W], f32)

            # zero the 1-col borders of racc (cols 0 and 97)
            nc.gpsimd.memset(racc[:, :, :, 0:RW:RW - 1], 0.0)
            # loads in first-use order
            ys3 = ysb[:].rearrange("p (r w) -> p r w", r=BLK)
            nc.sync.dma_start(out=ys3[0:64, 0:6, :], in_=yh[:, 0:6, :])
            nc.sync.dma_start(out=ys3[64:128, 0:6, :], in_=yh[:, 24:30, :])
            nc.sync.dma_start(out=wgsb[:, 0:1152], in_=wg[:, 0:1152])
            nc.sync.dma_start(out=gbsb[:, :], in_=gb[:, :])
            nc.sync.dma_start(out=xpad[0:64, 0:16, :], in_=xh[:, 0:16, :])
            nc.sync.dma_start(out=xpad[64:128, 0:16, :], in_=xh[:, 24:40, :])
            nc.sync.dma_start(out=wgsb[:, 1152:], in_=wg[:, 1152:])
            nc.sync.dma_start(out=ys3[0:64, 6:BLK, :], in_=yh[:, 6:BLK, :])
            nc.sync.dma_start(out=ys3[64:128, 6:BLK, :], in_=yh[:, 30:50, :])
            nc.sync.dma_start(out=xpad[0:64, 16:XR, :], in_=xh[:, 16:XR, :])
            nc.sync.dma_start(out=xpad[64:128, 16:XR, :], in_=xh[:, 40:24 + XR, :])
            nc.sync.dma_start(out=rmsb[:, :], in_=rm[:, :])
            nc.sync.dma_start(out=fwsb[:, 0:2304], in_=fw[:, 0:2304])
            nc.sync.dma_start(out=fwsb[:, 2304:], in_=fw[:, 2304:])
            nc.sync.dma_start(out=fbsb[:, :], in_=fb[:, :])

            # ---- fuse conv: per out-chunk, 36 accumulating matmuls
            # (4 groups x 9 taps; group g>0 gated on racc[g-1] row progress).
            # Pumped one matmul at a time between gen matmuls so the PE
            # never idles (idle gaps reset the p-state ramp).
            fuse_state = {}
            next_mm = {o0: 0 for (o0, _) in FCH}   # 0..36
            racc_done = [0, 0, 0]                  # kern rows complete per dil
            in_flight = []
            next_new = [0]

            def fuse_mm(o0):
                cnt = next_mm[o0]
                g, ij = cnt // 9, cnt % 9
                di, dj = ij // 3 - 1, ij % 3 - 1
                if cnt == 0:
                    fp = fpool.tile([128, 3 * W], f32, tag="fp")
                    fuse_state[o0] = fp
                fpv = fuse_state[o0][:].rearrange("p (r w) -> p r w", r=3)
                if g == 0:
                    rhs = xpad[:, o0 + di + 5:o0 + di + 8, 5 + dj:5 + dj + W]
                else:
                    rhs = racc[:, g - 1, o0 + di:o0 + di + 3, 1 + dj:1 + dj + W]
                nc.tensor.matmul(
                    fpv, fwsb[:, (ij * 4 + g) * 128:(ij * 4 + g + 1) * 128],
                    rhs, start=(cnt == 0), stop=(cnt == 35),
                )
                next_mm[o0] = cnt + 1
                if cnt == 35:
                    nc.scalar.activation(
                        osb[:, o0 - 1:o0 + 2, :], fpv, ident, bias=fbsb[:, 0:1])
                    del fuse_state[o0]
                    nc.sync.dma_start(out=out[:, o0 - 1:o0 + 2, :],
                                      in_=osb[0:64, o0 - 1:o0 + 2, :])
                    nc.sync.dma_start(out=out[:, 23 + o0:26 + o0, :],
                                      in_=osb[64:128, o0 - 1:o0 + 2, :])
                    in_flight.remove(o0)

            def pump_fuse(budget):
                while budget > 0:
                    for o0 in list(in_flight):
                        g = next_mm[o0] // 9
                        if g == 0 or racc_done[g - 1] >= o0 + 4:
                            fuse_mm(o0)
                            budget -= 1
                            break
                    else:
                        if next_new[0] < len(FCH) and len(in_flight) < 4:
                            in_flight.append(FCH[next_new[0]][0])
                            next_new[0] += 1
                            continue
                        return

            for ci, (r0, nrc) in enumerate(KCHUNKS):
                nh = nrc // 2  # rows per gen matmul (one PSUM bank each)
                for dd, d in enumerate(DILS):
                    pa = papool.tile([128, 9, 10, W], bf16, tag="pa")
                    sa = sapool.tile([128, 7, 10, W], bf16, tag="sa")
                    for t in range(9):
                        di, dj = t // 3 - 1, t % 3 - 1
                        dt = dd * 9 + t
                        kp = kpool.tile([128, 2, 512], f32, tag="kp")
                        for k in (0, 1):
                            nc.tensor.matmul(
                                kp[:, k, 0:nh * W],
                                wgsb[:, dt * 128:(dt + 1) * 128],
                                ysb[:, (r0 + k * nh) * W:(r0 + (k + 1) * nh) * W],
                                start=True, stop=True,
                            )
                        ks = kspool.tile([128, 10, W], bf16, tag="ks")
                        kpv = kp[:, :, 0:nh * W]
                        ksv = ks[:, 0:nrc, :].rearrange("p (b r) w -> p b (r w)", b=2)
                        if t in EV_POOL_TAPS:
                            nc.gpsimd.tensor_scalar(
                                ksv, kpv, gbsb[:, dt:dt + 1], None, add)
                        else:
                            nc.scalar.activation(
                                ksv, kpv, ident, bias=gbsb[:, dt:dt + 1])
                        x0 = r0 + di * d + 5
                        nc.vector.tensor_tensor(
                            pa[:, t, 0:nrc, :], ks[:, 0:nrc, :],
                            xpad[:, x0:x0 + nrc, 5 + dj * d:5 + dj * d + W], mult)
                        pump_fuse(4)
                    # 4-level pairwise tap-accumulation tree -> racc (bf16)
                    nc.gpsimd.tensor_tensor(
                        sa[:, 0:2, 0:nrc, :], pa[:, 0:4:2, 0:nrc, :],
                        pa[:, 1:4:2, 0:nrc, :], add)
                    nc.vector.tensor_tensor(
                        sa[:, 2:4, 0:nrc, :], pa[:, 4:8:2, 0:nrc, :],
                        pa[:, 5:8:2, 0:nrc, :], add)
                    nc.vector.tensor_tensor(
                        sa[:, 4, 0:nrc, :], sa[:, 0, 0:nrc, :],
                        sa[:, 1, 0:nrc, :], add)
                    nc.gpsimd.tensor_tensor(
                        sa[:, 5, 0:nrc, :], sa[:, 2, 0:nrc, :],
                        sa[:, 3, 0:nrc, :], add)
                    nc.vector.tensor_tensor(
                        sa[:, 6, 0:nrc, :], sa[:, 4, 0:nrc, :],
                        sa[:, 5, 0:nrc, :], add)
                    nc.vector.tensor_tensor(
                        racc[:, dd, r0:r0 + nrc, 1:1 + W], sa[:, 6, 0:nrc, :],
                        pa[:, 8, 0:nrc, :], add)
                    # zero out-of-image halo rows (reference zero-pads cat)
                    if r0 == 0:
                        nc.vector.tensor_scalar_mul(
                            racc[:, dd, 0, 1:1 + W], racc[:, dd, 0, 1:1 + W],
                            rmsb[:, 0:1])
                    elif r0 + nrc == BLK:
                        nc.vector.tensor_scalar_mul(
                            racc[:, dd, BLK - 1, 1:1 + W],
                            racc[:, dd, BLK - 1, 1:1 + W], rmsb[:, 1:2])
                    racc_done[dd] = r0 + nrc
                    pump_fuse(3)
            pump_fuse(10000)
    nc.finalize()
    return nc


def _prep_inputs(x, y, gen_w, gen_b, fuse_w, fuse_b):
    import ml_dtypes
    bf16 = ml_dtypes.bfloat16
    # generator weights: W_dt[c', c] = gen_w[d*576 + c*9 + t, c'],
    # block-diagonal over the two H-blocks.
    w3 = gen_w.reshape(3, 64, 9, 64).transpose(3, 0, 2, 1).reshape(64, 27, 64)
    wgh = np.zeros((128, 27, 128), np.float32)
    wgh[0:64, :, 0:64] = w3
    wgh[64:128, :, 64:128] = w3
    wgh = np.ascontiguousarray(wgh.reshape(128, 27 * 128))
    gbh = gen_b.reshape(3, 64, 9).transpose(1, 0, 2).reshape(64, 27)
    gbh = np.ascontiguousarray(np.concatenate([gbh, gbh], 0))
    # fuse weights: [k, ij, g, o] block-diagonal
    f3 = fuse_w.transpose(1, 2, 3, 0).reshape(4, 64, 9, 64).transpose(1, 2, 0, 3)
    fwh2 = np.zeros((128, 9, 4, 128), np.float32)
    fwh2[0:64, :, :, 0:64] = f3
    fwh2[64:128, :, :, 64:128] = f3
    fwh = np.ascontiguousarray(fwh2.reshape(128, 9 * 4 * 128).astype(bf16))
    fbh = np.ascontiguousarray(np.concatenate([fuse_b, fuse_b])[:, None])
    xp = np.pad(x, ((0, 0), (0, 0), (6, 6), (5, 5))).astype(bf16)
    yp = np.pad(y, ((0, 0), (0, 0), (1, 1), (0, 0)))
    maps = []
    for c in range(NCORES):
        n, half = c // 2, c % 2
        h0 = HH * half
        rmh = np.ones((128, 2), np.float32)
        if half == 0:
            rmh[0:64, 0] = 0.0     # block A row 0 = global row -1
        else:
            rmh[64:128, 1] = 0.0   # block B row 25 = global row 96
        maps.append({
            "xh": np.ascontiguousarray(xp[n, :, h0:h0 + 60, :]),
            "yh": np.ascontiguousarray(yp[n, :, h0:h0 + 50, :]),
            "wg": wgh, "gb": gbh, "fw": fwh, "fb": fbh, "rm": rmh,
        })
    return maps


def kernel(x, y, gen_w, gen_b, fuse_w, fuse_b):
    global _built
    from concourse.bass_utils import run_bass_kernel_spmd

    x = np.asarray(x, np.float32)
    y = np.asarray(y, np.float32)
    gen_w = np.asarray(gen_w, np.float32)
    gen_b = np.asarray(gen_b, np.float32)
    fuse_w = np.asarray(fuse_w, np.float32)
    fuse_b = np.asarray(fuse_b, np.float32)

    if _built is None:
        _built = _build()
    maps = _prep_inputs(x, y, gen_w, gen_b, fuse_w, fuse_b)
    res = run_bass_kernel_spmd(_built, maps, list(range(NCORES)))
    outf = np.empty((NB, C, H, W), np.float32)
    for c in range(NCORES):
        n, half = c // 2, c % 2
        outf[n, :, HH * half:HH * half + HH, :] = res.results[c]["out"]
    return outf


# revision 9
# speedup vs baseline: 1.1577x; 1.0795x over previous
"""Fused DDPM dynamic-conv kernel for TRN2 (8 NeuronCores).

Math (reference):
  kernels = einsum('nchw,oc->nohw', y, gen_w) + gen_b        # o = d*576 + c*9 + t
  r_d     = sum_t kernels[d,c,t] * shift(x, tap t, dil d)    # d in {1,3,5}
  out     = conv3x3([x, r1, r3, r5], fuse_w) + fuse_b

Sharding: 8 cores = 4 batches x 2 H-halves (48 output rows each).
Per core, the 50 kern rows (48 + 1 halo each side) are split into two
26-row blocks (2-row overlap) packed on SBUF partitions: p = 64*blk + c.

Engine split (vs. the all-PE/DVE baseline):
  PE  : gen matmuls (f32r) + fuse conv matmuls (bf16).  No identity
        tap-accumulation matmuls.
  ACT : evicts kern PSUM->SBUF bf16 with the gen bias fused
        (activation Identity + per-partition bias), and the fuse
        PSUM->SBUF eviction with fuse_b.
  DVE : tap products in bf16 (tensor_tensor mult runs in 2x mode for
        2-byte packed operands) + part of the add-tree.
  Pool: some evicts (taps 7,8) + part of the add-tree.
Tap accumulation is a 4-level pairwise tree in bf16; racc is bf16 so
the fuse matmuls stream it at 1 cycle/row like f32r.
"""

import numpy as np

K = 3
NB, C, H, W = 4, 64, 96, 96
NCORES = 8
HH = 48            # output rows per core
BLK = 26           # kern rows per block (24 out + 1 halo each side)
XR = BLK + 10      # x rows per block (halo 5 each side for dil 5)
WP = W + 10        # padded width for x
RW = W + 2         # padded width for racc
DILS = (1, 3, 5)
KCHUNKS = ((0, 6), (6, 10), (16, 10))   # kern-row chunks (start, nrows)
FCH = tuple((1 + 3 * i, 3) for i in range(8))  # fuse out-row chunks
EV_POOL_TAPS = (0, 4)  # evict units on Pool; rest on ACT

_built = None


def _build():
    import concourse.mybir as mybir
    from concourse import bacc
    from concourse.tile import TileContext

    f32 = mybir.dt.float32
    f32r = mybir.dt.float32r
    bf16 = mybir.dt.bfloat16
    add = mybir.AluOpType.add
    mult = mybir.AluOpType.mult
    ident = mybir.ActivationFunctionType.Identity

    nc = bacc.Bacc()
    xh = nc.dram_tensor("xh", [C, 60, WP], bf16, kind="ExternalInput")
    yh = nc.dram_tensor("yh", [C, 50, W], f32r, kind="ExternalInput")
    wg = nc.dram_tensor("wg", [128, 27 * 128], f32r, kind="ExternalInput")
    gb = nc.dram_tensor("gb", [128, 27], f32, kind="ExternalInput")
    fw = nc.dram_tensor("fw", [128, 9 * 4 * 128], bf16, kind="ExternalInput")
    fb = nc.dram_tensor("fb", [128, 1], f32, kind="ExternalInput")
    rm = nc.dram_tensor("rm", [128, 2], f32, kind="ExternalInput")
    out = nc.dram_tensor("out", [C, HH, W], f32, kind="ExternalOutput")

    with TileContext(nc) as tc:
        with (
            tc.tile_pool(name="const", bufs=1) as cpool,
            tc.tile_pool(name="ks", bufs=4) as kspool,
            tc.tile_pool(name="pa", bufs=2) as papool,
            tc.tile_pool(name="sa", bufs=2) as sapool,
            tc.tile_pool(name="kpsum", bufs=2, space="PSUM") as kpool,
            tc.tile_pool(name="fpsum", bufs=4, space="PSUM") as fpool,
        ):
            xpad = cpool.tile([128, XR, WP], bf16)
            ysb = cpool.tile([128, BLK * W], f32r)
            wgsb = cpool.tile([128, 27 * 128], f32r)
            gbsb = cpool.tile([128, 27], f32)
            fwsb = cpool.tile([128, 9 * 4 * 128], bf16)
            fbsb = cpool.tile([128, 1], f32)
            rmsb = cpool.tile([128, 2], f32)
            racc = cpool.tile([128, 3, BLK, RW], bf16)
            osb = cpool.tile([128, 24, W], f32)

            # zero the 1-col borders of racc (cols 0 and 97)
            nc.gpsimd.memset(racc[:, :, :, 0:RW:RW - 1], 0.0)
            # loads in first-use order
            ys3 = ysb[:].rearrange("p (r w) -> p r w", r=BLK)
            nc.sync.dma_start(out=ys3[0:64, 0:6, :], in_=yh[:, 0:6, :])
            nc.sync.dma_start(out=ys3[64:128, 0:6, :], in_=yh[:, 24:30, :])
            nc.sync.dma_start(out=wgsb[:, 0:256], in_=wg[:, 0:256])
            nc.sync.dma_start(out=wgsb[:, 256:1152], in_=wg[:, 256:1152])
            nc.sync.dma_start(out=gbsb[:, :], in_=gb[:, :])
            nc.sync.dma_start(out=xpad[0:64, 0:16, :], in_=xh[:, 0:16, :])
            nc.sync.dma_start(out=xpad[64:128, 0:16, :], in_=xh[:, 24:40, :])
            nc.sync.dma_start(out=wgsb[:, 1152:], in_=wg[:, 1152:])
            nc.sync.dma_start(out=ys3[0:64, 6:BLK, :], in_=yh[:, 6:BLK, :])
            nc.sync.dma_start(out=ys3[64:128, 6:BLK, :], in_=yh[:, 30:50, :])
            nc.sync.dma_start(out=xpad[0:64, 16:XR, :], in_=xh[:, 16:XR, :])
            nc.sync.dma_start(out=xpad[64:128, 16:XR, :], in_=xh[:, 40:24 + XR, :])
            nc.sync.dma_start(out=rmsb[:, :], in_=rm[:, :])
            nc.sync.dma_start(out=fwsb[:, 0:2304], in_=fw[:, 0:2304])
            nc.sync.dma_start(out=fwsb[:, 2304:], in_=fw[:, 2304:])
            nc.sync.dma_start(out=fbsb[:, :], in_=fb[:, :])

            # ---- fuse conv: per out-chunk, 36 accumulating matmuls
            # (4 groups x 9 taps; group g>0 gated on racc[g-1] row progress).
            # Pumped one matmul at a time between gen matmuls so the PE
            # never idles (idle gaps reset the p-state ramp).
            fuse_state = {}
            next_mm = {o0: 0 for (o0, _) in FCH}   # 0..36
            racc_done = [0, 0, 0]                  # kern rows complete per dil
            in_flight = []
            next_new = [0]

            def fuse_mm(o0):
                cnt = next_mm[o0]
                g, ij = cnt // 9, cnt % 9
                di, dj = ij // 3 - 1, ij % 3 - 1
                if cnt == 0:
                    fp = fpool.tile([128, 3 * W], f32, tag="fp")
                    fuse_state[o0] = fp
                fpv = fuse_state[o0][:].rearrange("p (r w) -> p r w", r=3)
                if g == 0:
                    rhs = xpad[:, o0 + di + 5:o0 + di + 8, 5 + dj:5 + dj + W]
                else:
                    rhs = racc[:, g - 1, o0 + di:o0 + di + 3, 1 + dj:1 + dj + W]
                nc.tensor.matmul(
                    fpv, fwsb[:, (ij * 4 + g) * 128:(ij * 4 + g + 1) * 128],
                    rhs, start=(cnt == 0), stop=(cnt == 35),
                )
                next_mm[o0] = cnt + 1
                if cnt == 35:
                    nc.scalar.activation(
                        osb[:, o0 - 1:o0 + 2, :], fpv, ident, bias=fbsb[:, 0:1])
                    del fuse_state[o0]
                    nc.sync.dma_start(out=out[:, o0 - 1:o0 + 2, :],
                                      in_=osb[0:64, o0 - 1:o0 + 2, :])
                    nc.sync.dma_start(out=out[:, 23 + o0:26 + o0, :],
                                      in_=osb[64:128, o0 - 1:o0 + 2, :])
                    in_flight.remove(o0)

            # racc_avail lags racc_done by one dil-group so a pumped fuse
            # matmul's racc semaphore has fired by the time the in-order PE
            # queue reaches it (the add-tree completes ~2us after its last
            # prod; a dil-group of gen matmuls is ~6us).
            racc_avail = [0, 0, 0]

            def pump_fuse(budget):
                while budget > 0:
                    for o0 in list(in_flight):
                        g = next_mm[o0] // 9
                        if g == 0 or racc_avail[g - 1] >= o0 + 4:
                            fuse_mm(o0)
                            budget -= 1
                            break
                    else:
                        if next_new[0] < len(FCH) and len(in_flight) < 4:
                            in_flight.append(FCH[next_new[0]][0])
                            next_new[0] += 1
                            continue
                        return

            for ci, (r0, nrc) in enumerate(KCHUNKS):
                nh = nrc // 2  # rows per gen matmul (one PSUM bank each)
                for dd, d in enumerate(DILS):
                    racc_avail[:] = racc_done
                    pa = papool.tile([128, 9, 10, W], bf16, tag="pa")
                    sa = sapool.tile([128, 7, 10, W], bf16, tag="sa")
                    for t in range(9):
                        di, dj = t // 3 - 1, t % 3 - 1
                        dt = dd * 9 + t
                        kp = kpool.tile([128, 2, 512], f32, tag="kp")
                        for k in (0, 1):
                            nc.tensor.matmul(
                                kp[:, k, 0:nh * W],
                                wgsb[:, dt * 128:(dt + 1) * 128],
                                ysb[:, (r0 + k * nh) * W:(r0 + (k + 1) * nh) * W],
                                start=True, stop=True,
                            )
                        ks = kspool.tile([128, 10, W], bf16, tag="ks")
                        kpv = kp[:, :, 0:nh * W]
                        ksv = ks[:, 0:nrc, :].rearrange("p (b r) w -> p b (r w)", b=2)
                        if t in EV_POOL_TAPS:
                            nc.gpsimd.tensor_scalar(
                                ksv, kpv, gbsb[:, dt:dt + 1], None, add)
                        else:
                            nc.scalar.activation(
                                ksv, kpv, ident, bias=gbsb[:, dt:dt + 1])
                        x0 = r0 + di * d + 5
                        nc.vector.tensor_tensor(
                            pa[:, t, 0:nrc, :], ks[:, 0:nrc, :],
                            xpad[:, x0:x0 + nrc, 5 + dj * d:5 + dj * d + W], mult)
                        # add-tree ops staged mid-group so only sa5/sa6/A4
                        # trail the last prod; Pool's ops run off-chain
                        if t == 4:
                            nc.gpsimd.tensor_tensor(
                                sa[:, 0:2, 0:nrc, :], pa[:, 0:4:2, 0:nrc, :],
                                pa[:, 1:4:2, 0:nrc, :], add)
                            nc.gpsimd.tensor_tensor(
                                sa[:, 4, 0:nrc, :], sa[:, 0, 0:nrc, :],
                                sa[:, 1, 0:nrc, :], add)
                        elif t == 8:
                            nc.vector.tensor_tensor(
                                sa[:, 2:4, 0:nrc, :], pa[:, 4:8:2, 0:nrc, :],
                                pa[:, 5:8:2, 0:nrc, :], add)
                        pump_fuse(4)
                    nc.vector.tensor_tensor(
                        sa[:, 5, 0:nrc, :], sa[:, 2, 0:nrc, :],
                        sa[:, 3, 0:nrc, :], add)
                    nc.vector.tensor_tensor(
                        sa[:, 6, 0:nrc, :], sa[:, 4, 0:nrc, :],
                        sa[:, 5, 0:nrc, :], add)
                    nc.vector.tensor_tensor(
                        racc[:, dd, r0:r0 + nrc, 1:1 + W], sa[:, 6, 0:nrc, :],
                        pa[:, 8, 0:nrc, :], add)
                    # zero out-of-image halo rows (reference zero-pads cat)
                    if r0 == 0:
                        nc.vector.tensor_scalar_mul(
                            racc[:, dd, 0, 1:1 + W], racc[:, dd, 0, 1:1 + W],
                            rmsb[:, 0:1])
                    elif r0 + nrc == BLK:
                        nc.vector.tensor_scalar_mul(
                            racc[:, dd, BLK - 1, 1:1 + W],
                            racc[:, dd, BLK - 1, 1:1 + W], rmsb[:, 1:2])
                    racc_done[dd] = r0 + nrc
                    pump_fuse(3)
            racc_avail[:] = racc_done
            pump_fuse(10000)
    nc.finalize()
    return nc


def _prep_inputs(x, y, gen_w, gen_b, fuse_w, fuse_b):
    import ml_dtypes
    bf16 = ml_dtypes.bfloat16
    # generator weights: W_dt[c', c] = gen_w[d*576 + c*9 + t, c'],
    # block-diagonal over the two H-blocks.
    w3 = gen_w.reshape(3, 64, 9, 64).transpose(3, 0, 2, 1).reshape(64, 27, 64)
    wgh = np.zeros((128, 27, 128), np.float32)
    wgh[0:64, :, 0:64] = w3
    wgh[64:128, :, 64:128] = w3
    wgh = np.ascontiguousarray(wgh.reshape(128, 27 * 128))
    gbh = gen_b.reshape(3, 64, 9).transpose(1, 0, 2).reshape(64, 27)
    gbh = np.ascontiguousarray(np.concatenate([gbh, gbh], 0))
    # fuse weights: [k, ij, g, o] block-diagonal
    f3 = fuse_w.transpose(1, 2, 3, 0).reshape(4, 64, 9, 64).transpose(1, 2, 0, 3)
    fwh2 = np.zeros((128, 9, 4, 128), np.float32)
    fwh2[0:64, :, :, 0:64] = f3
    fwh2[64:128, :, :, 64:128] = f3
    fwh = np.ascontiguousarray(fwh2.reshape(128, 9 * 4 * 128).astype(bf16))
    fbh = np.ascontiguousarray(np.concatenate([fuse_b, fuse_b])[:, None])
    xp = np.pad(x, ((0, 0), (0, 0), (6, 6), (5, 5))).astype(bf16)
    yp = np.pad(y, ((0, 0), (0, 0), (1, 1), (0, 0)))
    maps = []
    for c in range(NCORES):
        n, half = c // 2, c % 2
        h0 = HH * half
        rmh = np.ones((128, 2), np.float32)
        if half == 0:
            rmh[0:64, 0] = 0.0     # block A row 0 = global row -1
        else:
            rmh[64:128, 1] = 0.0   # block B row 25 = global row 96
        maps.append({
            "xh": np.ascontiguousarray(xp[n, :, h0:h0 + 60, :]),
            "yh": np.ascontiguousarray(yp[n, :, h0:h0 + 50, :]),
            "wg": wgh, "gb": gbh, "fw": fwh, "fb": fbh, "rm": rmh,
        })
    return maps


def kernel(x, y, gen_w, gen_b, fuse_w, fuse_b):
    global _built
    from concourse.bass_utils import run_bass_kernel_spmd

    x = np.asarray(x, np.float32)
    y = np.asarray(y, np.float32)
    gen_w = np.asarray(gen_w, np.float32)
    gen_b = np.asarray(gen_b, np.float32)
    fuse_w = np.asarray(fuse_w, np.float32)
    fuse_b = np.asarray(fuse_b, np.float32)

    if _built is None:
        _built = _build()
    maps = _prep_inputs(x, y, gen_w, gen_b, fuse_w, fuse_b)
    res = run_bass_kernel_spmd(_built, maps, list(range(NCORES)))
    outf = np.empty((NB, C, H, W), np.float32)
    for c in range(NCORES):
        n, half = c // 2, c % 2
        outf[n, :, HH * half:HH * half + HH, :] = res.results[c]["out"]
    return outf


# revision 11
# speedup vs baseline: 1.3182x; 1.1386x over previous
"""Fused DDPM dynamic-conv kernel for TRN2 (8 NeuronCores).

Math (reference):
  kernels = einsum('nchw,oc->nohw', y, gen_w) + gen_b        # o = d*576 + c*9 + t
  r_d     = sum_t kernels[d,c,t] * shift(x, tap t, dil d)    # d in {1,3,5}
  out     = conv3x3([x, r1, r3, r5], fuse_w) + fuse_b

Sharding: 8 cores = 4 batches x 2 H-halves (48 output rows each).
Per core, the 50 kern rows (48 + 1 halo each side) are split into two
26-row blocks (2-row overlap) packed on SBUF partitions: p = 64*blk + c.

Engine split (vs. the all-PE/DVE baseline):
  PE  : gen matmuls (f32r) + fuse conv matmuls (bf16).  No identity
        tap-accumulation matmuls.
  ACT : evicts kern PSUM->SBUF bf16 with the gen bias fused
        (activation Identity + per-partition bias), and the fuse
        PSUM->SBUF eviction with fuse_b.
  DVE : tap products in bf16 (tensor_tensor mult runs in 2x mode for
        2-byte packed operands) + part of the add-tree.
  Pool: some evicts (taps 7,8) + part of the add-tree.
Tap accumulation is a 4-level pairwise tree in bf16; racc is bf16 so
the fuse matmuls stream it at 1 cycle/row like f32r.
"""

import numpy as np

K = 3
NB, C, H, W = 4, 64, 96, 96
NCORES = 8
HH = 48            # output rows per core
BLK = 26           # kern rows per block (24 out + 1 halo each side)
XR = BLK + 10      # x rows per block (halo 5 each side for dil 5)
WP = W + 10        # padded width for x
RW = W + 2         # padded width for racc
DILS = (1, 3, 5)
KCHUNKS = ((0, 6), (6, 10), (16, 10))   # kern-row chunks (start, nrows)
FCH = tuple((1 + 3 * i, 3) for i in range(8))  # fuse out-row chunks
EV_POOL_TAPS = (0, 4)  # evict units on Pool; rest on ACT

_built = None


def _build():
    import concourse.mybir as mybir
    from concourse import bacc
    from concourse.tile import TileContext

    f32 = mybir.dt.float32
    f32r = mybir.dt.float32r
    bf16 = mybir.dt.bfloat16
    add = mybir.AluOpType.add
    mult = mybir.AluOpType.mult
    ident = mybir.ActivationFunctionType.Identity

    nc = bacc.Bacc()
    xh = nc.dram_tensor("xh", [C, 60, WP], bf16, kind="ExternalInput")
    yh = nc.dram_tensor("yh", [C, 50, W], f32r, kind="ExternalInput")
    wg = nc.dram_tensor("wg", [128, 27 * 128], f32r, kind="ExternalInput")
    gb = nc.dram_tensor("gb", [128, 27], f32, kind="ExternalInput")
    fw = nc.dram_tensor("fw", [128, 9 * 4 * 128], bf16, kind="ExternalInput")
    fb = nc.dram_tensor("fb", [128, 1], f32, kind="ExternalInput")
    rm = nc.dram_tensor("rm", [128, 2], f32, kind="ExternalInput")
    out = nc.dram_tensor("out", [C, HH, W], f32, kind="ExternalOutput")

    with TileContext(nc) as tc:
        with (
            tc.tile_pool(name="const", bufs=1) as cpool,
            tc.tile_pool(name="ks", bufs=6) as kspool,
            tc.tile_pool(name="pa", bufs=2) as papool,
            tc.tile_pool(name="sa", bufs=2) as sapool,
            tc.tile_pool(name="kpsum", bufs=3, space="PSUM") as kpool,
            tc.tile_pool(name="fpsum", bufs=2, space="PSUM") as fpool,
        ):
            xpad = cpool.tile([128, XR, WP], bf16)
            ysb = cpool.tile([128, BLK * W], f32r)
            wgsb = cpool.tile([128, 27 * 128], f32r)
            gbsb = cpool.tile([128, 27], f32)
            fwsb = cpool.tile([128, 9 * 4 * 128], bf16)
            fbsb = cpool.tile([128, 1], f32)
            rmsb = cpool.tile([128, 2], f32)
            racc = cpool.tile([128, 3, BLK, RW], bf16)
            osb = cpool.tile([128, 24, W], f32)

            # zero the 1-col borders of racc (cols 0 and 97)
            nc.gpsimd.memset(racc[:, :, :, 0:RW:RW - 1], 0.0)
            # loads in first-use order
            ys3 = ysb[:].rearrange("p (r w) -> p r w", r=BLK)
            nc.sync.dma_start(out=ys3[0:64, 0:6, :], in_=yh[:, 0:6, :])
            nc.sync.dma_start(out=ys3[64:128, 0:6, :], in_=yh[:, 24:30, :])
            nc.sync.dma_start(out=wgsb[:, 0:256], in_=wg[:, 0:256])
            nc.sync.dma_start(out=wgsb[:, 256:1152], in_=wg[:, 256:1152])
            nc.sync.dma_start(out=gbsb[:, :], in_=gb[:, :])
            nc.sync.dma_start(out=xpad[0:64, 0:16, :], in_=xh[:, 0:16, :])
            nc.sync.dma_start(out=xpad[64:128, 0:16, :], in_=xh[:, 24:40, :])
            nc.sync.dma_start(out=wgsb[:, 1152:], in_=wg[:, 1152:])
            nc.sync.dma_start(out=ys3[0:64, 6:BLK, :], in_=yh[:, 6:BLK, :])
            nc.sync.dma_start(out=ys3[64:128, 6:BLK, :], in_=yh[:, 30:50, :])
            nc.sync.dma_start(out=xpad[0:64, 16:XR, :], in_=xh[:, 16:XR, :])
            nc.sync.dma_start(out=xpad[64:128, 16:XR, :], in_=xh[:, 40:24 + XR, :])
            nc.sync.dma_start(out=rmsb[:, :], in_=rm[:, :])
            nc.sync.dma_start(out=fwsb[:, 0:2304], in_=fw[:, 0:2304])
            nc.sync.dma_start(out=fwsb[:, 2304:], in_=fw[:, 2304:])
            nc.sync.dma_start(out=fbsb[:, :], in_=fb[:, :])

            # ---- fuse conv: per out-chunk, 36 accumulating matmuls
            # (4 groups x 9 taps; group g>0 gated on racc[g-1] row progress).
            # Pumped one matmul at a time between gen matmuls so the PE
            # never idles (idle gaps reset the p-state ramp).
            fuse_state = {}
            next_mm = {o0: 0 for (o0, _) in FCH}   # 0..36
            racc_done = [0, 0, 0]                  # kern rows complete per dil
            in_flight = []
            next_new = [0]

            def fuse_mm(o0):
                cnt = next_mm[o0]
                g, ij = cnt // 9, cnt % 9
                di, dj = ij // 3 - 1, ij % 3 - 1
                if cnt == 0:
                    fp = fpool.tile([128, 3 * W], f32, tag="fp")
                    fuse_state[o0] = fp
                fpv = fuse_state[o0][:].rearrange("p (r w) -> p r w", r=3)
                if g == 0:
                    rhs = xpad[:, o0 + di + 5:o0 + di + 8, 5 + dj:5 + dj + W]
                else:
                    rhs = racc[:, g - 1, o0 + di:o0 + di + 3, 1 + dj:1 + dj + W]
                nc.tensor.matmul(
                    fpv, fwsb[:, (ij * 4 + g) * 128:(ij * 4 + g + 1) * 128],
                    rhs, start=(cnt == 0), stop=(cnt == 35),
                )
                next_mm[o0] = cnt + 1
                if cnt == 35:
                    nc.scalar.activation(
                        osb[:, o0 - 1:o0 + 2, :], fpv, ident, bias=fbsb[:, 0:1])
                    del fuse_state[o0]
                    nc.sync.dma_start(out=out[:, o0 - 1:o0 + 2, :],
                                      in_=osb[0:64, o0 - 1:o0 + 2, :])
                    nc.sync.dma_start(out=out[:, 23 + o0:26 + o0, :],
                                      in_=osb[64:128, o0 - 1:o0 + 2, :])
                    in_flight.remove(o0)

            # racc_avail lags racc_done by one dil-group so a pumped fuse
            # matmul's racc semaphore has fired by the time the in-order PE
            # queue reaches it (the add-tree completes ~2us after its last
            # prod; a dil-group of gen matmuls is ~6us).
            racc_avail = [0, 0, 0]

            def pump_fuse(budget):
                while budget > 0:
                    for o0 in list(in_flight):
                        g = next_mm[o0] // 9
                        if g == 0 or racc_avail[g - 1] >= o0 + 4:
                            fuse_mm(o0)
                            budget -= 1
                            break
                    else:
                        if next_new[0] < len(FCH) and len(in_flight) < 2:
                            in_flight.append(FCH[next_new[0]][0])
                            next_new[0] += 1
                            continue
                        return

            for ci, (r0, nrc) in enumerate(KCHUNKS):
                nh = nrc // 2  # rows per gen matmul (one PSUM bank each)
                for dd, d in enumerate(DILS):
                    racc_avail[:] = racc_done
                    pa = papool.tile([128, 9, 10, W], bf16, tag="pa")
                    sa = sapool.tile([128, 7, 10, W], bf16, tag="sa")
                    for t in range(9):
                        di, dj = t // 3 - 1, t % 3 - 1
                        dt = dd * 9 + t
                        kp = kpool.tile([128, 2, 512], f32, tag="kp")
                        for k in (0, 1):
                            nc.tensor.matmul(
                                kp[:, k, 0:nh * W],
                                wgsb[:, dt * 128:(dt + 1) * 128],
                                ysb[:, (r0 + k * nh) * W:(r0 + (k + 1) * nh) * W],
                                start=True, stop=True,
                            )
                        x0 = r0 + di * d + 5
                        xv = xpad[:, x0:x0 + nrc, 5 + dj * d:5 + dj * d + W
                                  ].rearrange("p (b r) w -> p b r w", r=nh)
                        pav = pa[:, t, 0:nrc, :].rearrange(
                            "p (b r) w -> p b r w", r=nh)
                        kpv3 = kp[:, :, 0:nh * W].rearrange(
                            "p b (r w) -> p b r w", w=W)
                        if t in EV_POOL_TAPS:
                            # Pool cannot read PSUM: fuse bias+product on DVE
                            nc.vector.scalar_tensor_tensor(
                                pav, kpv3, gbsb[:, dt:dt + 1], xv, add, mult)
                        else:
                            ks = kspool.tile([128, 10, W], bf16, tag="ks")
                            ksv = ks[:, 0:nrc, :].rearrange(
                                "p (b r) w -> p b (r w)", b=2)
                            nc.scalar.activation(
                                ksv, kp[:, :, 0:nh * W], ident,
                                bias=gbsb[:, dt:dt + 1])
                            nc.vector.tensor_tensor(
                                pa[:, t, 0:nrc, :], ks[:, 0:nrc, :],
                                xpad[:, x0:x0 + nrc,
                                     5 + dj * d:5 + dj * d + W], mult)
                        # add-tree ops staged mid-group so only sa5/sa6/A4
                        # trail the last prod; Pool's ops run off-chain
                        if t == 4:
                            nc.gpsimd.tensor_tensor(
                                sa[:, 0:2, 0:nrc, :], pa[:, 0:4:2, 0:nrc, :],
                                pa[:, 1:4:2, 0:nrc, :], add)
                            nc.gpsimd.tensor_tensor(
                                sa[:, 4, 0:nrc, :], sa[:, 0, 0:nrc, :],
                                sa[:, 1, 0:nrc, :], add)
                        elif t == 8:
                            nc.vector.tensor_tensor(
                                sa[:, 2:4, 0:nrc, :], pa[:, 4:8:2, 0:nrc, :],
                                pa[:, 5:8:2, 0:nrc, :], add)
                        pump_fuse(4)
                    nc.vector.tensor_tensor(
                        sa[:, 5, 0:nrc, :], sa[:, 2, 0:nrc, :],
                        sa[:, 3, 0:nrc, :], add)
                    nc.vector.tensor_tensor(
                        sa[:, 6, 0:nrc, :], sa[:, 4, 0:nrc, :],
                        sa[:, 5, 0:nrc, :], add)
                    nc.vector.tensor_tensor(
                        racc[:, dd, r0:r0 + nrc, 1:1 + W], sa[:, 6, 0:nrc, :],
                        pa[:, 8, 0:nrc, :], add)
                    # zero out-of-image halo rows (reference zero-pads cat)
                    if r0 == 0:
                        nc.vector.tensor_scalar_mul(
                            racc[:, dd, 0, 1:1 + W], racc[:, dd, 0, 1:1 + W],
                            rmsb[:, 0:1])
                    elif r0 + nrc == BLK:
                        nc.vector.tensor_scalar_mul(
                            racc[:, dd, BLK - 1, 1:1 + W],
                            racc[:, dd, BLK - 1, 1:1 + W], rmsb[:, 1:2])
                    racc_done[dd] = r0 + nrc
                    pump_fuse(3)
            racc_avail[:] = racc_done
            pump_fuse(10000)
    nc.finalize()
    return nc


def _prep_inputs(x, y, gen_w, gen_b, fuse_w, fuse_b):
    import ml_dtypes
    bf16 = ml_dtypes.bfloat16
    # generator weights: W_dt[c', c] = gen_w[d*576 + c*9 + t, c'],
    # block-diagonal over the two H-blocks.
    w3 = gen_w.reshape(3, 64, 9, 64).transpose(3, 0, 2, 1).reshape(64, 27, 64)
    wgh = np.zeros((128, 27, 128), np.float32)
    wgh[0:64, :, 0:64] = w3
    wgh[64:128, :, 64:128] = w3
    wgh = np.ascontiguousarray(wgh.reshape(128, 27 * 128))
    gbh = gen_b.reshape(3, 64, 9).transpose(1, 0, 2).reshape(64, 27)
    gbh = np.ascontiguousarray(np.concatenate([gbh, gbh], 0))
    # fuse weights: [k, ij, g, o] block-diagonal
    f3 = fuse_w.transpose(1, 2, 3, 0).reshape(4, 64, 9, 64).transpose(1, 2, 0, 3)
    fwh2 = np.zeros((128, 9, 4, 128), np.float32)
    fwh2[0:64, :, :, 0:64] = f3
    fwh2[64:128, :, :, 64:128] = f3
    fwh = np.ascontiguousarray(fwh2.reshape(128, 9 * 4 * 128).astype(bf16))
    fbh = np.ascontiguousarray(np.concatenate([fuse_b, fuse_b])[:, None])
    xp = np.pad(x, ((0, 0), (0, 0), (6, 6), (5, 5))).astype(bf16)
    yp = np.pad(y, ((0, 0), (0, 0), (1, 1), (0, 0)))
    maps = []
    for c in range(NCORES):
        n, half = c // 2, c % 2
        h0 = HH * half
        rmh = np.ones((128, 2), np.float32)
        if half == 0:
            rmh[0:64, 0] = 0.0     # block A row 0 = global row -1
        else:
            rmh[64:128, 1] = 0.0   # block B row 25 = global row 96
        maps.append({
            "xh": np.ascontiguousarray(xp[n, :, h0:h0 + 60, :]),
            "yh": np.ascontiguousarray(yp[n, :, h0:h0 + 50, :]),
            "wg": wgh, "gb": gbh, "fw": fwh, "fb": fbh, "rm": rmh,
        })
    return maps


def kernel(x, y, gen_w, gen_b, fuse_w, fuse_b):
    global _built
    from concourse.bass_utils import run_bass_kernel_spmd

    x = np.asarray(x, np.float32)
    y = np.asarray(y, np.float32)
    gen_w = np.asarray(gen_w, np.float32)
    gen_b = np.asarray(gen_b, np.float32)
    fuse_w = np.asarray(fuse_w, np.float32)
    fuse_b = np.asarray(fuse_b, np.float32)

    if _built is None:
        _built = _build()
    maps = _prep_inputs(x, y, gen_w, gen_b, fuse_w, fuse_b)
    res = run_bass_kernel_spmd(_built, maps, list(range(NCORES)))
    outf = np.empty((NB, C, H, W), np.float32)
    for c in range(NCORES):
        n, half = c // 2, c % 2
        outf[n, :, HH * half:HH * half + HH, :] = res.results[c]["out"]
    return outf


# revision 15
# speedup vs baseline: 1.4390x; 1.0917x over previous
"""Fused DDPM dynamic-conv kernel for TRN2 (8 NeuronCores).

Math (reference):
  kernels = einsum('nchw,oc->nohw', y, gen_w) + gen_b        # o = d*576 + c*9 + t
  r_d     = sum_t kernels[d,c,t] * shift(x, tap t, dil d)    # d in {1,3,5}
  out     = conv3x3([x, r1, r3, r5], fuse_w) + fuse_b

Sharding: 8 cores = 4 batches x 2 H-halves (48 output rows each).
Per core, the 50 kern rows (48 + 1 halo each side) are split into two
26-row blocks (2-row overlap) packed on SBUF partitions: p = 64*blk + c.

Engine split (vs. the all-PE/DVE baseline):
  PE  : gen matmuls (f32r) + fuse conv matmuls (bf16).  No identity
        tap-accumulation matmuls.
  ACT : evicts kern PSUM->SBUF bf16 with the gen bias fused
        (activation Identity + per-partition bias), and the fuse
        PSUM->SBUF eviction with fuse_b.
  DVE : tap products in bf16 (tensor_tensor mult runs in 2x mode for
        2-byte packed operands) + part of the add-tree.
  Pool: some evicts (taps 7,8) + part of the add-tree.
Tap accumulation is a 4-level pairwise tree in bf16; racc is bf16 so
the fuse matmuls stream it at 1 cycle/row like f32r.
"""

import numpy as np

K = 3
NB, C, H, W = 4, 64, 96, 96
NCORES = 8
HH = 48            # output rows per core
BLK = 26           # kern rows per block (24 out + 1 halo each side)
XR = BLK + 10      # x rows per block (halo 5 each side for dil 5)
WP = W + 10        # padded width for x
RW = W + 2         # padded width for racc
DILS = (1, 3, 5)
KCHUNKS = ((0, 6), (6, 10), (16, 10))   # kern-row chunks (start, nrows)
FCH = tuple((1 + 3 * i, 3) for i in range(8))  # fuse out-row chunks
EV_POOL_TAPS = (0, 4)  # evict units on Pool; rest on ACT

_built = None


def _build():
    import concourse.mybir as mybir
    from concourse import bacc
    from concourse.tile import TileContext

    f32 = mybir.dt.float32
    f32r = mybir.dt.float32r
    bf16 = mybir.dt.bfloat16
    add = mybir.AluOpType.add
    mult = mybir.AluOpType.mult
    ident = mybir.ActivationFunctionType.Identity

    nc = bacc.Bacc()
    xh = nc.dram_tensor("xh", [C, 60, WP], bf16, kind="ExternalInput")
    yh = nc.dram_tensor("yh", [C, 50, W], f32r, kind="ExternalInput")
    wg = nc.dram_tensor("wg", [128, 27 * 128], f32r, kind="ExternalInput")
    gb = nc.dram_tensor("gb", [128, 27], f32, kind="ExternalInput")
    fw = nc.dram_tensor("fw", [128, 9 * 4 * 128], bf16, kind="ExternalInput")
    fb = nc.dram_tensor("fb", [128, 1], f32, kind="ExternalInput")
    rm = nc.dram_tensor("rm", [128, 2], f32, kind="ExternalInput")
    out = nc.dram_tensor("out", [C, HH, W], f32, kind="ExternalOutput")

    with TileContext(nc) as tc:
        with (
            tc.tile_pool(name="const", bufs=1) as cpool,
            tc.tile_pool(name="ks", bufs=6) as kspool,
            tc.tile_pool(name="pa", bufs=2) as papool,
            tc.tile_pool(name="sa", bufs=2) as sapool,
            tc.tile_pool(name="kpsum", bufs=3, space="PSUM") as kpool,
            tc.tile_pool(name="fpsum", bufs=2, space="PSUM") as fpool,
        ):
            xpad = cpool.tile([128, XR, WP], bf16)
            ysb = cpool.tile([128, BLK * W], f32r)
            wgsb = cpool.tile([128, 27 * 128], f32r)
            gbsb = cpool.tile([128, 27], f32)
            fwsb = cpool.tile([128, 9 * 4 * 128], bf16)
            fbsb = cpool.tile([128, 1], f32)
            rmsb = cpool.tile([128, 2], f32)
            racc = cpool.tile([128, 3, BLK, RW], bf16)
            osb = cpool.tile([128, 24, W], f32)

            # zero the 1-col borders of racc (cols 0 and 97)
            nc.gpsimd.memset(racc[:, :, :, 0:RW:RW - 1], 0.0)
            # loads in first-use order
            ys3 = ysb[:].rearrange("p (r w) -> p r w", r=BLK)
            nc.sync.dma_start(out=ys3[0:64, 0:6, :], in_=yh[:, 0:6, :])
            nc.sync.dma_start(out=ys3[64:128, 0:6, :], in_=yh[:, 24:30, :])
            nc.sync.dma_start(out=wgsb[:, 0:256], in_=wg[:, 0:256])
            nc.sync.dma_start(out=wgsb[:, 256:1152], in_=wg[:, 256:1152])
            nc.sync.dma_start(out=gbsb[:, :], in_=gb[:, :])
            nc.sync.dma_start(out=xpad[0:64, 0:16, :], in_=xh[:, 0:16, :])
            nc.sync.dma_start(out=xpad[64:128, 0:16, :], in_=xh[:, 24:40, :])
            nc.sync.dma_start(out=wgsb[:, 1152:], in_=wg[:, 1152:])
            nc.sync.dma_start(out=ys3[0:64, 6:BLK, :], in_=yh[:, 6:BLK, :])
            nc.sync.dma_start(out=ys3[64:128, 6:BLK, :], in_=yh[:, 30:50, :])
            nc.sync.dma_start(out=xpad[0:64, 16:XR, :], in_=xh[:, 16:XR, :])
            nc.sync.dma_start(out=xpad[64:128, 16:XR, :], in_=xh[:, 40:24 + XR, :])
            nc.sync.dma_start(out=rmsb[:, :], in_=rm[:, :])
            nc.sync.dma_start(out=fwsb[:, 0:2304], in_=fw[:, 0:2304])
            nc.sync.dma_start(out=fwsb[:, 2304:], in_=fw[:, 2304:])
            nc.sync.dma_start(out=fbsb[:, :], in_=fb[:, :])

            # ---- fuse conv: per out-chunk, 36 accumulating matmuls
            # (4 groups x 9 taps; group g>0 gated on racc[g-1] row progress).
            # Pumped one matmul at a time between gen matmuls so the PE
            # never idles (idle gaps reset the p-state ramp).
            fuse_state = {}
            next_mm = {o0: 0 for (o0, _) in FCH}   # 0..36
            racc_done = [0, 0, 0]                  # kern rows complete per dil
            in_flight = []
            next_new = [0]

            def fuse_mm(o0):
                cnt = next_mm[o0]
                g, ij = cnt // 9, cnt % 9
                di, dj = ij // 3 - 1, ij % 3 - 1
                if cnt == 0:
                    fp = fpool.tile([128, 3 * W], f32, tag="fp")
                    fuse_state[o0] = fp
                fpv = fuse_state[o0][:].rearrange("p (r w) -> p r w", r=3)
                if g == 0:
                    rhs = xpad[:, o0 + di + 5:o0 + di + 8, 5 + dj:5 + dj + W]
                else:
                    rhs = racc[:, g - 1, o0 + di:o0 + di + 3, 1 + dj:1 + dj + W]
                nc.tensor.matmul(
                    fpv, fwsb[:, (ij * 4 + g) * 128:(ij * 4 + g + 1) * 128],
                    rhs, start=(cnt == 0), stop=(cnt == 35),
                )
                next_mm[o0] = cnt + 1
                if cnt == 35:
                    nc.scalar.activation(
                        osb[:, o0 - 1:o0 + 2, :], fpv, ident, bias=fbsb[:, 0:1])
                    del fuse_state[o0]
                    nc.sync.dma_start(out=out[:, o0 - 1:o0 + 2, :],
                                      in_=osb[0:64, o0 - 1:o0 + 2, :])
                    nc.sync.dma_start(out=out[:, 23 + o0:26 + o0, :],
                                      in_=osb[64:128, o0 - 1:o0 + 2, :])
                    in_flight.remove(o0)

            # racc_avail lags racc_done by one dil-group so a pumped fuse
            # matmul's racc semaphore has fired by the time the in-order PE
            # queue reaches it (the add-tree completes ~2us after its last
            # prod; a dil-group of gen matmuls is ~6us).
            racc_avail = [0, 0, 0]

            def pump_fuse(budget):
                while budget > 0:
                    for o0 in list(in_flight):
                        g = next_mm[o0] // 9
                        if g == 0 or racc_avail[g - 1] >= o0 + 4:
                            fuse_mm(o0)
                            budget -= 1
                            break
                    else:
                        if next_new[0] < len(FCH) and len(in_flight) < 2:
                            in_flight.append(FCH[next_new[0]][0])
                            next_new[0] += 1
                            continue
                        return

            def dil_group(r0, nrc, dd, d):
                    # rows per gen matmul: 2 PSUM-bank segments unless the
                    # segment would fall under the 256-col f32r speed floor
                    nh = nrc // 2
                    seg, rh = (2, nh) if nh * W >= 256 else (1, nrc)
                    racc_avail[:] = racc_done
                    pa = papool.tile([128, 9, 10, W], bf16, tag="pa")
                    sa = sapool.tile([128, 7, 10, W], bf16, tag="sa")
                    for t in range(9):
                        di, dj = t // 3 - 1, t % 3 - 1
                        dt = dd * 9 + t
                        kp = kpool.tile([128, 2, 512], f32, tag="kp")
                        for k in range(seg):
                            nc.tensor.matmul(
                                kp[:, k, 0:rh * W],
                                wgsb[:, dt * 128:(dt + 1) * 128],
                                ysb[:, (r0 + k * rh) * W:(r0 + (k + 1) * rh) * W],
                                start=True, stop=True,
                            )
                        x0 = r0 + di * d + 5
                        xv = xpad[:, x0:x0 + nrc, 5 + dj * d:5 + dj * d + W
                                  ].rearrange("p (b r) w -> p b r w", r=rh)
                        pav = pa[:, t, 0:nrc, :].rearrange(
                            "p (b r) w -> p b r w", r=rh)
                        kpv3 = kp[:, 0:seg, 0:rh * W].rearrange(
                            "p b (r w) -> p b r w", w=W)
                        if t in EV_POOL_TAPS:
                            # Pool cannot read PSUM: fuse bias+product on DVE
                            nc.vector.scalar_tensor_tensor(
                                pav, kpv3, gbsb[:, dt:dt + 1], xv, add, mult)
                        else:
                            ks = kspool.tile([128, 10, W], bf16, tag="ks")
                            ksv = ks[:, 0:nrc, :].rearrange(
                                "p (b r) w -> p b (r w)", b=seg)
                            nc.scalar.activation(
                                ksv, kp[:, 0:seg, 0:rh * W], ident,
                                bias=gbsb[:, dt:dt + 1])
                            eng = nc.gpsimd if t == 1 else nc.vector
                            eng.tensor_tensor(
                                pa[:, t, 0:nrc, :], ks[:, 0:nrc, :],
                                xpad[:, x0:x0 + nrc,
                                     5 + dj * d:5 + dj * d + W], mult)
                        # add-tree ops staged mid-group so only sa5/sa6/A4
                        # trail the last prod; Pool's ops run off-chain
                        if t == 4:
                            nc.gpsimd.tensor_tensor(
                                sa[:, 0:2, 0:nrc, :], pa[:, 0:4:2, 0:nrc, :],
                                pa[:, 1:4:2, 0:nrc, :], add)
                            nc.gpsimd.tensor_tensor(
                                sa[:, 4, 0:nrc, :], sa[:, 0, 0:nrc, :],
                                sa[:, 1, 0:nrc, :], add)
                        elif t == 8:
                            nc.vector.tensor_tensor(
                                sa[:, 2:4, 0:nrc, :], pa[:, 4:8:2, 0:nrc, :],
                                pa[:, 5:8:2, 0:nrc, :], add)
                        pump_fuse(4)
                    nc.vector.tensor_tensor(
                        sa[:, 5, 0:nrc, :], sa[:, 2, 0:nrc, :],
                        sa[:, 3, 0:nrc, :], add)
                    nc.vector.tensor_tensor(
                        sa[:, 6, 0:nrc, :], sa[:, 4, 0:nrc, :],
                        sa[:, 5, 0:nrc, :], add)
                    nc.vector.tensor_tensor(
                        racc[:, dd, r0:r0 + nrc, 1:1 + W], sa[:, 6, 0:nrc, :],
                        pa[:, 8, 0:nrc, :], add)
                    # zero out-of-image halo rows (reference zero-pads cat)
                    if r0 == 0:
                        nc.vector.tensor_scalar_mul(
                            racc[:, dd, 0, 1:1 + W], racc[:, dd, 0, 1:1 + W],
                            rmsb[:, 0:1])
                    elif r0 + nrc == BLK:
                        nc.vector.tensor_scalar_mul(
                            racc[:, dd, BLK - 1, 1:1 + W],
                            racc[:, dd, BLK - 1, 1:1 + W], rmsb[:, 1:2])
                    racc_done[dd] = r0 + nrc
                    pump_fuse(3)

            for ci, (r0, nrc) in enumerate(KCHUNKS):
                for dd, d in enumerate(DILS):
                    if ci == 2 and dd == 2:
                        # split the final dil-group so racc[2] row progress
                        # unlocks fuse chunks 13/16 before the tail flush
                        dil_group(16, 6, dd, d)
                        dil_group(22, 4, dd, d)
                    else:
                        dil_group(r0, nrc, dd, d)
            racc_avail[:] = racc_done
            pump_fuse(10000)
    nc.finalize()
    return nc


def _prep_inputs(x, y, gen_w, gen_b, fuse_w, fuse_b):
    import ml_dtypes
    bf16 = ml_dtypes.bfloat16
    # generator weights: W_dt[c', c] = gen_w[d*576 + c*9 + t, c'],
    # block-diagonal over the two H-blocks.
    w3 = gen_w.reshape(3, 64, 9, 64).transpose(3, 0, 2, 1).reshape(64, 27, 64)
    wgh = np.zeros((128, 27, 128), np.float32)
    wgh[0:64, :, 0:64] = w3
    wgh[64:128, :, 64:128] = w3
    wgh = np.ascontiguousarray(wgh.reshape(128, 27 * 128))
    gbh = gen_b.reshape(3, 64, 9).transpose(1, 0, 2).reshape(64, 27)
    gbh = np.ascontiguousarray(np.concatenate([gbh, gbh], 0))
    # fuse weights: [k, ij, g, o] block-diagonal
    f3 = fuse_w.transpose(1, 2, 3, 0).reshape(4, 64, 9, 64).transpose(1, 2, 0, 3)
    fwh2 = np.zeros((128, 9, 4, 128), np.float32)
    fwh2[0:64, :, :, 0:64] = f3
    fwh2[64:128, :, :, 64:128] = f3
    fwh = np.ascontiguousarray(fwh2.reshape(128, 9 * 4 * 128).astype(bf16))
    fbh = np.ascontiguousarray(np.concatenate([fuse_b, fuse_b])[:, None])
    xp = np.pad(x, ((0, 0), (0, 0), (6, 6), (5, 5))).astype(bf16)
    yp = np.pad(y, ((0, 0), (0, 0), (1, 1), (0, 0)))
    maps = []
    for c in range(NCORES):
        n, half = c // 2, c % 2
        h0 = HH * half
        rmh = np.ones((128, 2), np.float32)
        if half == 0:
            rmh[0:64, 0] = 0.0     # block A row 0 = global row -1
        else:
            rmh[64:128, 1] = 0.0   # block B row 25 = global row 96
        maps.append({
            "xh": np.ascontiguousarray(xp[n, :, h0:h0 + 60, :]),
            "yh": np.ascontiguousarray(yp[n, :, h0:h0 + 50, :]),
            "wg": wgh, "gb": gbh, "fw": fwh, "fb": fbh, "rm": rmh,
        })
    return maps


def kernel(x, y, gen_w, gen_b, fuse_w, fuse_b):
    global _built
    from concourse.bass_utils import run_bass_kernel_spmd

    x = np.asarray(x, np.float32)
    y = np.asarray(y, np.float32)
    gen_w = np.asarray(gen_w, np.float32)
    gen_b = np.asarray(gen_b, np.float32)
    fuse_w = np.asarray(fuse_w, np.float32)
    fuse_b = np.asarray(fuse_b, np.float32)

    if _built is None:
        _built = _build()
    maps = _prep_inputs(x, y, gen_w, gen_b, fuse_w, fuse_b)
    res = run_bass_kernel_spmd(_built, maps, list(range(NCORES)))
    outf = np.empty((NB, C, H, W), np.float32)
    for c in range(NCORES):
        n, half = c // 2, c % 2
        outf[n, :, HH * half:HH * half + HH, :] = res.results[c]["out"]
    return outf


# revision 20
# speedup vs baseline: 1.4800x; 1.0284x over previous
"""Fused DDPM dynamic-conv kernel for TRN2 (8 NeuronCores).

Math (reference):
  kernels = einsum('nchw,oc->nohw', y, gen_w) + gen_b        # o = d*576 + c*9 + t
  r_d     = sum_t kernels[d,c,t] * shift(x, tap t, dil d)    # d in {1,3,5}
  out     = conv3x3([x, r1, r3, r5], fuse_w) + fuse_b

Sharding: 8 cores = 4 batches x 2 H-halves (48 output rows each).
Per core, the 50 kern rows (48 + 1 halo each side) are split into two
26-row blocks (2-row overlap) packed on SBUF partitions: p = 64*blk + c.

Engine split (vs. the all-PE/DVE baseline):
  PE  : gen matmuls (f32r) + fuse conv matmuls (bf16).  No identity
        tap-accumulation matmuls.
  ACT : evicts kern PSUM->SBUF bf16 with the gen bias fused
        (activation Identity + per-partition bias), and the fuse
        PSUM->SBUF eviction with fuse_b.
  DVE : tap products in bf16 (tensor_tensor mult runs in 2x mode for
        2-byte packed operands) + part of the add-tree.
  Pool: some evicts (taps 7,8) + part of the add-tree.
Tap accumulation is a 4-level pairwise tree in bf16; racc is bf16 so
the fuse matmuls stream it at 1 cycle/row like f32r.
"""

import numpy as np

K = 3
NB, C, H, W = 4, 64, 96, 96
NCORES = 8
HH = 48            # output rows per core
BLK = 26           # kern rows per block (24 out + 1 halo each side)
XR = BLK + 10      # x rows per block (halo 5 each side for dil 5)
WP = W + 10        # padded width for x
RW = W + 2         # padded width for racc
DILS = (1, 3, 5)
KCHUNKS = ((0, 6), (6, 10), (16, 10))   # kern-row chunks (start, nrows)
FCH = tuple((1 + 3 * i, 3) for i in range(8))  # fuse out-row chunks
EV_POOL_TAPS = (3, 7)  # evict units on Pool; rest on ACT

_built = None


def _build():
    import concourse.mybir as mybir
    from concourse import bacc
    from concourse.tile import TileContext

    f32 = mybir.dt.float32
    f32r = mybir.dt.float32r
    bf16 = mybir.dt.bfloat16
    add = mybir.AluOpType.add
    mult = mybir.AluOpType.mult
    ident = mybir.ActivationFunctionType.Identity

    nc = bacc.Bacc()
    xh = nc.dram_tensor("xh", [C, 60, WP], bf16, kind="ExternalInput")
    yh = nc.dram_tensor("yh", [C, 50, W], f32r, kind="ExternalInput")
    wg = nc.dram_tensor("wg", [128, 27 * 128], f32r, kind="ExternalInput")
    gb = nc.dram_tensor("gb", [128, 27], f32, kind="ExternalInput")
    fw = nc.dram_tensor("fw", [128, 9 * 4 * 128], bf16, kind="ExternalInput")
    fb = nc.dram_tensor("fb", [128, 1], f32, kind="ExternalInput")
    rm = nc.dram_tensor("rm", [128, 2], f32, kind="ExternalInput")
    out = nc.dram_tensor("out", [C, HH, W], f32, kind="ExternalOutput")

    with TileContext(nc) as tc:
        with (
            tc.tile_pool(name="const", bufs=1) as cpool,
            tc.tile_pool(name="ks", bufs=6) as kspool,
            tc.tile_pool(name="pa", bufs=2) as papool,
            tc.tile_pool(name="sa", bufs=2) as sapool,
            tc.tile_pool(name="kpsum", bufs=3, space="PSUM") as kpool,
            tc.tile_pool(name="fpsum", bufs=2, space="PSUM") as fpool,
        ):
            xpad = cpool.tile([128, XR, WP], bf16)
            ysb = cpool.tile([128, BLK * W], f32r)
            wgsb = cpool.tile([128, 27 * 128], f32r)
            gbsb = cpool.tile([128, 27], f32)
            fwsb = cpool.tile([128, 9 * 4 * 128], bf16)
            fbsb = cpool.tile([128, 1], f32)
            rmsb = cpool.tile([128, 2], f32)
            racc = cpool.tile([128, 3, BLK, RW], bf16)
            osb = cpool.tile([128, 24, W], f32)

            # zero the 1-col borders of racc (cols 0 and 97)
            nc.gpsimd.memset(racc[:, :, :, 0:RW:RW - 1], 0.0)
            # loads in first-use order
            ys3 = ysb[:].rearrange("p (r w) -> p r w", r=BLK)
            nc.sync.dma_start(out=ys3[0:64, 0:6, :], in_=yh[:, 0:6, :])
            nc.sync.dma_start(out=ys3[64:128, 0:6, :], in_=yh[:, 24:30, :])
            nc.sync.dma_start(out=wgsb[:, 0:256], in_=wg[:, 0:256])
            nc.sync.dma_start(out=wgsb[:, 256:1152], in_=wg[:, 256:1152])
            nc.sync.dma_start(out=gbsb[:, :], in_=gb[:, :])
            nc.sync.dma_start(out=xpad[0:64, 0:16, :], in_=xh[:, 0:16, :])
            nc.sync.dma_start(out=xpad[64:128, 0:16, :], in_=xh[:, 24:40, :])
            nc.sync.dma_start(out=wgsb[:, 1152:], in_=wg[:, 1152:])
            nc.sync.dma_start(out=ys3[0:64, 6:BLK, :], in_=yh[:, 6:BLK, :])
            nc.sync.dma_start(out=ys3[64:128, 6:BLK, :], in_=yh[:, 30:50, :])
            nc.sync.dma_start(out=xpad[0:64, 16:XR, :], in_=xh[:, 16:XR, :])
            nc.sync.dma_start(out=xpad[64:128, 16:XR, :], in_=xh[:, 40:24 + XR, :])
            nc.sync.dma_start(out=rmsb[:, :], in_=rm[:, :])
            nc.sync.dma_start(out=fwsb[:, 0:2304], in_=fw[:, 0:2304])
            nc.sync.dma_start(out=fwsb[:, 2304:], in_=fw[:, 2304:])
            nc.sync.dma_start(out=fbsb[:, :], in_=fb[:, :])

            # ---- fuse conv: per out-chunk, 36 accumulating matmuls
            # (4 groups x 9 taps; group g>0 gated on racc[g-1] row progress).
            # Pumped one matmul at a time between gen matmuls so the PE
            # never idles (idle gaps reset the p-state ramp).
            fuse_state = {}
            next_mm = {o0: 0 for (o0, _) in FCH}   # 0..36
            racc_done = [0, 0, 0]                  # kern rows complete per dil
            in_flight = []
            next_new = [0]

            def fuse_mm(o0):
                cnt = next_mm[o0]
                g, ij = cnt // 9, cnt % 9
                di, dj = ij // 3 - 1, ij % 3 - 1
                if cnt == 0:
                    fp = fpool.tile([128, 3 * W], f32, tag="fp")
                    fuse_state[o0] = fp
                fpv = fuse_state[o0][:].rearrange("p (r w) -> p r w", r=3)
                if g == 0:
                    rhs = xpad[:, o0 + di + 5:o0 + di + 8, 5 + dj:5 + dj + W]
                else:
                    rhs = racc[:, g - 1, o0 + di:o0 + di + 3, 1 + dj:1 + dj + W]
                nc.tensor.matmul(
                    fpv, fwsb[:, (ij * 4 + g) * 128:(ij * 4 + g + 1) * 128],
                    rhs, start=(cnt == 0), stop=(cnt == 35),
                )
                next_mm[o0] = cnt + 1
                if cnt == 35:
                    nc.scalar.activation(
                        osb[:, o0 - 1:o0 + 2, :], fpv, ident, bias=fbsb[:, 0:1])
                    del fuse_state[o0]
                    nc.sync.dma_start(out=out[:, o0 - 1:o0 + 2, :],
                                      in_=osb[0:64, o0 - 1:o0 + 2, :])
                    nc.sync.dma_start(out=out[:, 23 + o0:26 + o0, :],
                                      in_=osb[64:128, o0 - 1:o0 + 2, :])
                    in_flight.remove(o0)

            # racc_avail lags racc_done by one dil-group so a pumped fuse
            # matmul's racc semaphore has fired by the time the in-order PE
            # queue reaches it (the add-tree completes ~2us after its last
            # prod; a dil-group of gen matmuls is ~6us).
            racc_avail = [0, 0, 0]

            def pump_fuse(budget):
                while budget > 0:
                    for o0 in list(in_flight):
                        g = next_mm[o0] // 9
                        if g == 0 or racc_avail[g - 1] >= o0 + 4:
                            fuse_mm(o0)
                            budget -= 1
                            break
                    else:
                        if next_new[0] < len(FCH) and len(in_flight) < 2:
                            in_flight.append(FCH[next_new[0]][0])
                            next_new[0] += 1
                            continue
                        return

            def dil_group(r0, nrc, dd, d):
                    # rows per gen matmul: 2 PSUM-bank segments unless the
                    # segment would fall under the 256-col f32r speed floor
                    nh = nrc // 2
                    seg, rh = (2, nh) if nh * W >= 256 else (1, nrc)
                    racc_avail[:] = racc_done
                    pa = papool.tile([128, 9, 10, W], bf16, tag="pa")
                    sa = sapool.tile([128, 7, 10, W], bf16, tag="sa")
                    for t in range(9):
                        di, dj = t // 3 - 1, t % 3 - 1
                        dt = dd * 9 + t
                        kp = kpool.tile([128, 2, 512], f32, tag="kp")
                        for k in range(seg):
                            nc.tensor.matmul(
                                kp[:, k, 0:rh * W],
                                wgsb[:, dt * 128:(dt + 1) * 128],
                                ysb[:, (r0 + k * rh) * W:(r0 + (k + 1) * rh) * W],
                                start=True, stop=True,
                            )
                        x0 = r0 + di * d + 5
                        xv = xpad[:, x0:x0 + nrc, 5 + dj * d:5 + dj * d + W
                                  ].rearrange("p (b r) w -> p b r w", r=rh)
                        pav = pa[:, t, 0:nrc, :].rearrange(
                            "p (b r) w -> p b r w", r=rh)
                        kpv3 = kp[:, 0:seg, 0:rh * W].rearrange(
                            "p b (r w) -> p b r w", w=W)
                        if t in EV_POOL_TAPS:
                            # Pool cannot read PSUM: fuse bias+product on DVE
                            nc.vector.scalar_tensor_tensor(
                                pav, kpv3, gbsb[:, dt:dt + 1], xv, add, mult)
                        else:
                            ks = kspool.tile([128, 10, W], bf16, tag="ks")
                            ksv = ks[:, 0:nrc, :].rearrange(
                                "p (b r) w -> p b (r w)", b=seg)
                            nc.scalar.activation(
                                ksv, kp[:, 0:seg, 0:rh * W], ident,
                                bias=gbsb[:, dt:dt + 1])
                            eng = nc.gpsimd if t == 1 else nc.vector
                            eng.tensor_tensor(
                                pa[:, t, 0:nrc, :], ks[:, 0:nrc, :],
                                xpad[:, x0:x0 + nrc,
                                     5 + dj * d:5 + dj * d + W], mult)
                        # add-tree ops staged mid-group so only sa5/sa6/A4
                        # trail the last prod; Pool's ops run off-chain
                        if t == 4:
                            nc.gpsimd.tensor_tensor(
                                sa[:, 0:2, 0:nrc, :], pa[:, 0:4:2, 0:nrc, :],
                                pa[:, 1:4:2, 0:nrc, :], add)
                            nc.gpsimd.tensor_tensor(
                                sa[:, 4, 0:nrc, :], sa[:, 0, 0:nrc, :],
                                sa[:, 1, 0:nrc, :], add)
                        elif t == 8:
                            nc.vector.tensor_tensor(
                                sa[:, 2:4, 0:nrc, :], pa[:, 4:8:2, 0:nrc, :],
                                pa[:, 5:8:2, 0:nrc, :], add)
                        pump_fuse(4)
                    nc.vector.tensor_tensor(
                        sa[:, 5, 0:nrc, :], sa[:, 2, 0:nrc, :],
                        sa[:, 3, 0:nrc, :], add)
                    nc.vector.tensor_tensor(
                        sa[:, 6, 0:nrc, :], sa[:, 4, 0:nrc, :],
                        sa[:, 5, 0:nrc, :], add)
                    nc.vector.tensor_tensor(
                        racc[:, dd, r0:r0 + nrc, 1:1 + W], sa[:, 6, 0:nrc, :],
                        pa[:, 8, 0:nrc, :], add)
                    # zero out-of-image halo rows (reference zero-pads cat)
                    if r0 == 0:
                        nc.vector.tensor_scalar_mul(
                            racc[:, dd, 0, 1:1 + W], racc[:, dd, 0, 1:1 + W],
                            rmsb[:, 0:1])
                    elif r0 + nrc == BLK:
                        nc.vector.tensor_scalar_mul(
                            racc[:, dd, BLK - 1, 1:1 + W],
                            racc[:, dd, BLK - 1, 1:1 + W], rmsb[:, 1:2])
                    racc_done[dd] = r0 + nrc
                    pump_fuse(3)

            for ci, (r0, nrc) in enumerate(KCHUNKS):
                for dd, d in enumerate(DILS):
                    if ci == 2 and dd == 2:
                        # split the final dil-group so racc[2] row progress
                        # unlocks fuse chunks 13/16 before the tail flush
                        dil_group(16, 6, dd, d)
                        dil_group(22, 4, dd, d)
                    else:
                        dil_group(r0, nrc, dd, d)
            racc_avail[:] = racc_done
            pump_fuse(10000)
    nc.finalize()
    return nc


def _prep_inputs(x, y, gen_w, gen_b, fuse_w, fuse_b):
    import ml_dtypes
    bf16 = ml_dtypes.bfloat16
    # generator weights: W_dt[c', c] = gen_w[d*576 + c*9 + t, c'],
    # block-diagonal over the two H-blocks.
    w3 = gen_w.reshape(3, 64, 9, 64).transpose(3, 0, 2, 1).reshape(64, 27, 64)
    wgh = np.zeros((128, 27, 128), np.float32)
    wgh[0:64, :, 0:64] = w3
    wgh[64:128, :, 64:128] = w3
    wgh = np.ascontiguousarray(wgh.reshape(128, 27 * 128))
    gbh = gen_b.reshape(3, 64, 9).transpose(1, 0, 2).reshape(64, 27)
    gbh = np.ascontiguousarray(np.concatenate([gbh, gbh], 0))
    # fuse weights: [k, ij, g, o] block-diagonal
    f3 = fuse_w.transpose(1, 2, 3, 0).reshape(4, 64, 9, 64).transpose(1, 2, 0, 3)
    fwh2 = np.zeros((128, 9, 4, 128), np.float32)
    fwh2[0:64, :, :, 0:64] = f3
    fwh2[64:128, :, :, 64:128] = f3
    fwh = np.ascontiguousarray(fwh2.reshape(128, 9 * 4 * 128).astype(bf16))
    fbh = np.ascontiguousarray(np.concatenate([fuse_b, fuse_b])[:, None])
    xp = np.pad(x, ((0, 0), (0, 0), (6, 6), (5, 5))).astype(bf16)
    yp = np.pad(y, ((0, 0), (0, 0), (1, 1), (0, 0)))
    maps = []
    for c in range(NCORES):
        n, half = c // 2, c % 2
        h0 = HH * half
        rmh = np.ones((128, 2), np.float32)
        if half == 0:
            rmh[0:64, 0] = 0.0     # block A row 0 = global row -1
        else:
            rmh[64:128, 1] = 0.0   # block B row 25 = global row 96
        maps.append({
            "xh": np.ascontiguousarray(xp[n, :, h0:h0 + 60, :]),
            "yh": np.ascontiguousarray(yp[n, :, h0:h0 + 50, :]),
            "wg": wgh, "gb": gbh, "fw": fwh, "fb": fbh, "rm": rmh,
        })
    return maps


def kernel(x, y, gen_w, gen_b, fuse_w, fuse_b):
    global _built
    from concourse.bass_utils import run_bass_kernel_spmd

    x = np.asarray(x, np.float32)
    y = np.asarray(y, np.float32)
    gen_w = np.asarray(gen_w, np.float32)
    gen_b = np.asarray(gen_b, np.float32)
    fuse_w = np.asarray(fuse_w, np.float32)
    fuse_b = np.asarray(fuse_b, np.float32)

    if _built is None:
        _built = _build()
    maps = _prep_inputs(x, y, gen_w, gen_b, fuse_w, fuse_b)
    res = run_bass_kernel_spmd(_built, maps, list(range(NCORES)))
    outf = np.empty((NB, C, H, W), np.float32)
    for c in range(NCORES):
        n, half = c // 2, c % 2
        outf[n, :, HH * half:HH * half + HH, :] = res.results[c]["out"]
    return outf
